# revision 21
# baseline (speedup 1.0000x reference)
"""Trainium2 Bass kernel for a single-head attention layer.

Problem: x [4, 2048, 1024] f32; torch-Linear qkv (W_qkv [3072, 1024]) ->
single-head attention (d=1024) -> output projection (W_proj [1024, 1024]).

Sharding: 8 NeuronCores = 4 batches x 2 query-halves. Each core computes
attention for 1024 queries of one batch. K^T/V are computed only for the
core's own 1024-key half; the partner half is exchanged through pairwise
AllGather collectives (replica groups [[0,1],[2,3],[4,5],[6,7]]). While the
collectives are in flight the core projects Q and runs attention over its own
keys; the partner half is imported from the gathered DRAM buffer with a
partition-id-derived dynamic row offset (rank parity picks the block).
Per-core key order is [own half, partner half] - softmax is permutation-
invariant over keys, so this is safe.

Host-side folds (all exact or fp32):
  - 1/sqrt(d) scale folded into W_q / b_q
  - V-bias folded through the projection: b_eff = b_proj + W_proj @ b_v
  - softmax normalization on host: the device returns unnormalized
    y^T = W_proj @ (exp(scores^T) @ V)^T plus per-query exp-sums.
    No max-subtraction needed: logits are ~N(0,1), exp is safe in f32.

DMA strategy: every input is pre-packed on the host into its exact SBUF
image [128, 8192] so each dma_start moves a [128, 2048] slice with fully
contiguous 4KB rows (the sweet spot for per-stream DGE throughput; one
dma_start fans its packets across all 16 DMA engines). This cuts the input
load from 33 issue-serialized 1KB-row dma_starts to 17 4KB-row ones.

K^T is computed in two contraction passes (d0-3 -> f32 stash via the scalar
engine with the K-bias folded in, then d4-7 + stash via a DVE add) so the
tensor engine starts after only 2MB of input has landed instead of 4MB.

Device program (per core; all matmuls bf16 with f32 PSUM accumulation):
  A: K^T_own[e,s] -> SBUF + send; V_own[s,e] -> SBUF + send; Q^T[e,q]
  B: scores^T[j,q] = K^T-stationary x Q^T-moving -> Exp -> SBUF bf16;
     per-q sums via ones-column matmul (keys on partitions)
  C: out^T[d,q] = V-stationary x exp^T-moving (accumulated over key tiles)
  D: y^T[e,q] = W_proj^T-stationary x out^T-moving -> bf16 out
"""

import math

import numpy as np
import ml_dtypes

import concourse.bass as bass
import concourse.tile as tile
from concourse import mybir
from concourse.bass_utils import run_bass_kernel_spmd
from concourse.vector_clock import ScopedClock, VectorClock

BF16 = mybir.dt.bfloat16
F32 = mybir.dt.float32
AF = mybir.ActivationFunctionType

D = 1024   # model dim
S = 2048   # sequence length
Q = 1024   # queries per core
H = 1024   # keys per core (own half)
P = 128    # SBUF partitions
NB = 512   # matmul moving-block size
DT = D // P
HT = H // P
ST = S // P
N_CORES = 8
GROUPS = [[0, 1], [2, 3], [4, 5], [6, 7]]
W = D * DT          # 8192: packed image width
LQ = 2048           # dma_start slice width (4KB bf16 rows)

# ---------------------------------------------------------------------------
# Workarounds for this container's walrus, which rejects any instruction
# carrying more than one sem wait ("Too many sync wait commands").
# ---------------------------------------------------------------------------


def _patched_drain_and_barrier(self, tick_clock, wait_clock):
    # Split the kernel-tail drain into one drain per semaphore (1 wait each).
    gc = tick_clock.global_clock
    n = len(gc)
    for i in range(n):
        if gc[i] > 0:
            vec = [0] * n
            vec[i] = gc[i]
            dr = self.nc.sync.drain()
            wait_clock.add_sem_waits(dr.ins, ScopedClock({None: VectorClock(vec)}))
    self.nc.all_engine_barrier()
    popped = self.nc._tile_sem_poison_stack.pop()
    assert popped is self._sem_poison
    self.nc.clear_and_free_semaphores(list(self.sems.allocated().values()))
    self.nc.all_engine_barrier()


_MAX_WAITS = 1
_split_counter = [0]


def _split_excess_waits(ordered):
    # Hoist excess waits onto preceding same-engine NoOps.
    for insts in ordered.values():
        new_list = []
        for inst in insts:
            si = inst.sync_info
            waits = list(si.on_wait) if si is not None and si.on_wait else []
            if len(waits) > _MAX_WAITS and inst.engine is not None:
                extra, keep = waits[:-_MAX_WAITS], waits[-_MAX_WAITS:]
                for w in extra:
                    _split_counter[0] += 1
                    nop = mybir.InstNoOp(
                        name=f"waitsplit-{_split_counter[0]}",
                        sync_info=mybir.SyncInfo(on_wait=[w], on_update=[]),
                        bass_nofuse=True,
                        engine=inst.engine,
                    )
                    new_list.append(nop)
                inst.sync_info = mybir.SyncInfo(
                    on_wait=keep, on_update=list(si.on_update))
            new_list.append(inst)
        insts[:] = new_list


def _install_patches():
    if getattr(tile.TileContext, "_attn_patched", False):
        return
    tile.TileContext._drain_and_barrier = _patched_drain_and_barrier
    orig_lower = tile.TileContext._lower_ordered_insts

    def _lower_with_wait_split(self, ordered):
        _split_excess_waits(ordered)
        return orig_lower(self, ordered)

    tile.TileContext._lower_ordered_insts = _lower_with_wait_split
    tile.TileContext._attn_patched = True


_install_patches()

# ---------------------------------------------------------------------------
# Device program
# ---------------------------------------------------------------------------


def build_nc():
    nc = bass.Bass("TRN2", target_bir_lowering=False, debug=False,
                   num_devices=N_CORES)

    # Packed SBUF images: col block d*1024+c holds row d*128+p, col c of the
    # logical [1024, 1024] operand.
    xt = nc.dram_tensor("xt", [P, W], BF16, kind="ExternalInput").ap()
    wq = nc.dram_tensor("wq", [P, W], BF16, kind="ExternalInput").ap()
    wk = nc.dram_tensor("wk", [P, W], BF16, kind="ExternalInput").ap()
    wv = nc.dram_tensor("wv", [P, W], BF16, kind="ExternalInput").ap()
    wp = nc.dram_tensor("wp", [P, W], BF16, kind="ExternalInput").ap()
    bqk = nc.dram_tensor("bqk", [P, 2 * DT], F32, kind="ExternalInput").ap()
    yt = nc.dram_tensor("yt", [P, W], BF16, kind="ExternalOutput").ap()
    sums = nc.dram_tensor("sums", [1, Q], F32, kind="ExternalOutput").ap()

    # K is exchanged in two halves (e0-3, e4-7) so the partner import can
    # start as soon as the first half of K^T is computed.
    k_send = [nc.dram_tensor(f"k_send{h}", [D // 2, H], BF16).ap()
              for h in range(2)]
    k_recv = [nc.dram_tensor(f"k_recv{h}", [D, H], BF16).ap()
              for h in range(2)]
    v_send = nc.dram_tensor("v_send", [H, D], BF16).ap()
    v_recv = nc.dram_tensor("v_recv", [2 * H, D], BF16).ap()

    from contextlib import ExitStack
    with tile.TileContext(nc) as tc, ExitStack() as stack:
        res = stack.enter_context(tc.tile_pool(name="res", bufs=1))
        qt_t = [res.tile([P, Q], BF16, tag=f"qt{e}", name=f"qt{e}")
                for e in range(DT)]
        kt_own = [res.tile([P, H], BF16, tag=f"kto{e}", name=f"kto{e}")
                  for e in range(DT)]
        v_own = [res.tile([P, D], BF16, tag=f"vo{j}", name=f"vo{j}")
                 for j in range(HT)]
        bias_t = res.tile([P, 2 * DT], F32, tag="bias", name="bias")

        # ---------------- Phase A: projections + exchange ----------------
        # pha_ps uses 3 banks (+1 warmup) so phase B's pools land on virgin
        # PSUM banks - otherwise B's first matmul waits phase A's last drain.
        with tc.tile_pool(name="pha", bufs=1) as pha, \
             tc.tile_pool(name="pha_ps", bufs=3, space="PSUM") as pha_ps:
            xt_sb = pha.tile([P, W], BF16, tag="xt", name="xt_sb")
            wk_sb = pha.tile([P, W], BF16, tag="wk", name="wk_sb")
            wv_sb = pha.tile([P, W], BF16, tag="wv", name="wv_sb")
            wq_sb = pha.tile([P, W], BF16, tag="wq", name="wq_sb")
            # f32 stash for the partial K^T contractions (+ K-bias)
            stash = [pha.tile([P, NB], F32, tag=f"st{g}", name=f"st{g}")
                     for g in range(2 * DT)]
            # PE warmup: junk matmuls on a zeroed tile, gated only by a DVE
            # memset - they run during the input-DMA wait and ramp the tensor
            # clock to full p-state before the first real matmul.
            warm = pha.tile([P, NB], BF16, tag="warm", name="warm")
            nc.vector.memset(warm, 0.0)

            def qsl(t, q4):
                return t[:, q4 * LQ:(q4 + 1) * LQ]

            # Inputs split across two DGE queues so both rings spin up
            # concurrently and the 1MB pass-1 gate (xt q0 on sync, wk q0 on
            # scalar) issues ~1.5us earlier. wk fits on the scalar queue:
            # its 4 issues retire before phase A's first scalar drain.
            nc.sync.dma_start(out=bias_t, in_=bqk[:, :])
            for q4 in range(4):
                nc.scalar.dma_start(out=qsl(wk_sb, q4), in_=qsl(wk, q4))
            for q4 in range(4):
                nc.sync.dma_start(out=qsl(xt_sb, q4), in_=qsl(xt, q4))
            for q4 in range(4):
                nc.sync.dma_start(out=qsl(wv_sb, q4), in_=qsl(wv, q4))
            for q4 in range(4):
                nc.sync.dma_start(out=qsl(wq_sb, q4), in_=qsl(wq, q4))

            with tc.tile_pool(name="wm_ps", bufs=1, space="PSUM") as wm_pool:
                wm_ps = wm_pool.tile([P, NB], F32, tag="wm", name="wm_ps")
                for _ in range(14):
                    nc.tensor.matmul(wm_ps, warm[:, :P], warm,
                                     start=True, stop=True)
                nc.vector.tensor_copy(out=warm, in_=wm_ps)

            def xcol(d, c0, n):
                return xt_sb[:, d * D + c0:d * D + c0 + n]

            def wcol(t, d, c0, n):
                return t[:, d * D + c0:d * D + c0 + n]

            # K^T_own[e, s_own] in two contraction passes: d0-1 -> f32 stash
            # (+K-bias) so the PE starts after 1MB of input, then d2-7 +
            # stash -> bf16. Pass-1 drains alternate between the scalar and
            # DVE engines so the 2-matmul chains stay PE-bound.
            for g in range(2 * DT):
                e, sb = divmod(g, 2)
                ps = pha_ps.tile([P, NB], F32, tag="ps")
                for d in range(2):
                    nc.tensor.matmul(
                        ps, wcol(wk_sb, d, e * P, P),
                        xcol(d, sb * NB, NB),
                        start=(d == 0), stop=(d == 1))
                if g % 2 == 0:
                    nc.scalar.activation(
                        out=stash[g], in_=ps,
                        func=AF.Identity, bias=bias_t[:, DT + e:DT + e + 1])
                else:
                    nc.vector.tensor_scalar_add(
                        stash[g], ps, bias_t[:, DT + e:DT + e + 1])
            for h in range(2):
                for e in range(h * 4, h * 4 + 4):
                    for sb in range(2):
                        ps = pha_ps.tile([P, NB], F32, tag="ps")
                        for d in range(2, DT):
                            nc.tensor.matmul(
                                ps, wcol(wk_sb, d, e * P, P),
                                xcol(d, sb * NB, NB),
                                start=(d == 2), stop=(d == DT - 1))
                        nc.vector.tensor_add(
                            kt_own[e][:, sb * NB:(sb + 1) * NB],
                            ps, stash[e * 2 + sb])
                    nc.sync.dma_start(
                        out=k_send[h][(e - h * 4) * P:(e - h * 4 + 1) * P, :],
                        in_=kt_own[e])
                nc.gpsimd.collective_compute(
                    "AllGather", mybir.AluOpType.bypass,
                    replica_groups=GROUPS,
                    ins=[k_send[h][:, :]], outs=[k_recv[h][:, :]])

            # V_own[s_own, e] -> SBUF (kept) + v_send. V before Q so the V
            # AllGather triggers early enough for the phase-C j=8 boundary.
            for j in range(HT):
                for eb in range(D // NB):
                    ps = pha_ps.tile([P, NB], F32, tag="ps")
                    for d in range(DT):
                        nc.tensor.matmul(
                            ps, xcol(d, j * P, P),
                            wcol(wv_sb, d, eb * NB, NB),
                            start=(d == 0), stop=(d == DT - 1))
                    nc.vector.tensor_copy(
                        out=v_own[j][:, eb * NB:(eb + 1) * NB], in_=ps)
                nc.sync.dma_start(out=v_send[j * P:(j + 1) * P, :],
                                  in_=v_own[j])

            nc.gpsimd.collective_compute(
                "AllGather", mybir.AluOpType.bypass, replica_groups=GROUPS,
                ins=[v_send[:, :]], outs=[v_recv[:, :]])

            # Q^T[e, q] (overlaps the collectives)
            for e in range(DT):
                for qb in range(Q // NB):
                    ps = pha_ps.tile([P, NB], F32, tag="ps")
                    for d in range(DT):
                        nc.tensor.matmul(
                            ps, wcol(wq_sb, d, e * P, P),
                            xcol(d, qb * NB, NB),
                            start=(d == 0), stop=(d == DT - 1))
                    nc.scalar.activation(
                        out=qt_t[e][:, qb * NB:(qb + 1) * NB], in_=ps,
                        func=AF.Identity, bias=bias_t[:, e:e + 1])

        # ---------------- Phases B, C, D ----------------
        with tc.tile_pool(name="phb", bufs=1) as phb, \
             tc.tile_pool(name="scr_ps", bufs=2, space="PSUM") as scr_ps, \
             tc.tile_pool(name="sum_ps", bufs=1, space="PSUM") as sum_ps, \
             tc.tile_pool(name="av_ps", bufs=1, space="PSUM") as av_ps:
            # Partner-half import: rank parity picks the gathered block.
            # One dynamic-offset DMA per tensor (SP base registers are scarce).
            # Partner imports on the sync queue (behind the v_send stores).
            # The K AllGather halves complete long before the j=8 boundary;
            # the gpsimd queue is NOT usable here because collective_compute
            # blocks it until the collective completes.
            pid = nc.sync.partition_id()
            parity = pid % 2
            kbase = (1 - parity) * (D // 2)
            pbase = (1 - parity) * H
            kt_par = phb.tile([P, DT, H], BF16, tag="ktp", name="ktp")
            v_par = phb.tile([P, HT, D], BF16, tag="vp", name="vp")
            for h in range(2):
                nc.sync.dma_start(
                    out=kt_par[:, h * 4:(h + 1) * 4, :],
                    in_=k_recv[h][bass.ds(kbase, D // 2), :].rearrange(
                        "(e p) s -> p e s", p=P))
            nc.sync.dma_start(
                out=v_par,
                in_=v_recv[bass.ds(pbase, H), :].rearrange(
                    "(j p) e -> p j e", p=P))

            exp_t = [phb.tile([P, Q], BF16, tag=f"exp{j}", name=f"exp{j}")
                     for j in range(ST)]
            out_t = [phb.tile([P, Q], BF16, tag=f"out{j}", name=f"out{j}")
                     for j in range(DT)]
            wp_sb = phb.tile([P, W], BF16, tag="wp", name="wp_sb")
            for q4 in range(4):
                nc.sync.dma_start(out=wp_sb[:, q4 * LQ:(q4 + 1) * LQ],
                                  in_=wp[:, q4 * LQ:(q4 + 1) * LQ])

            # f32 per-key-lane partial sums, accumulated on the (idle) DVE;
            # reduced across partitions with two small f32 matmuls at the end.
            sumacc = phb.tile([P, Q], F32, tag="sumacc", name="sumacc")
            ones_f = phb.tile([P, 1], F32, tag="ones_f", name="ones_f")
            nc.vector.memset(ones_f, 1.0)

            def kt_slice(e, j):
                if j < HT:
                    return kt_own[e][:, j * P:(j + 1) * P]
                return kt_par[:, e, (j - HT) * P:(j - HT + 1) * P]

            def v_tile(j):
                if j < HT:
                    return v_own[j]
                return v_par[:, j - HT, :]

            # B: scores^T[j,q] -> exp -> sums (own keys first)
            for j in range(ST):
                for qb in range(Q // NB):
                    ps = scr_ps.tile([P, NB], F32, tag="scr")
                    for e in range(DT):
                        nc.tensor.matmul(
                            ps, kt_slice(e, j),
                            qt_t[e][:, qb * NB:(qb + 1) * NB],
                            start=(e == 0), stop=(e == DT - 1))
                    nc.scalar.activation(
                        out=exp_t[j][:, qb * NB:(qb + 1) * NB], in_=ps,
                        func=AF.Exp)
                    sl = slice(qb * NB, (qb + 1) * NB)
                    if j == 0:
                        nc.vector.tensor_copy(
                            out=sumacc[:, sl], in_=exp_t[j][:, sl])
                    else:
                        nc.vector.tensor_add(
                            sumacc[:, sl], sumacc[:, sl], exp_t[j][:, sl])

            # C: out^T[d',q] accumulated over all 16 key tiles. Each PSUM
            # bank runs its full j-sweep serially, so bank i's drain copy
            # overlaps bank i+1's sweep - no stall when the bank is reused.
            for g in range(2):
                for qb in range(Q // NB):
                    for i in range(4):
                        dp = g * 4 + i
                        ps_o = av_ps.tile([P, NB], F32, tag=f"av{i}",
                                          name=f"avps{i}")
                        for j in range(ST):
                            nc.tensor.matmul(
                                ps_o, v_tile(j)[:, dp * P:(dp + 1) * P],
                                exp_t[j][:, qb * NB:(qb + 1) * NB],
                                start=(j == 0), stop=(j == ST - 1))
                        nc.vector.tensor_copy(
                            out=out_t[dp][:, qb * NB:(qb + 1) * NB],
                            in_=ps_o)

            # Softmax denominators: reduced after phase C so the tiny fsum
            # matmuls never stall the B->C transition on the PE queue.
            sums_sb = phb.tile([1, Q], F32, tag="sums_sb")
            for qb in range(Q // NB):
                fs = sum_ps.tile([1, NB], F32, tag=f"fsum{qb}",
                                 name=f"fsum{qb}")
                nc.tensor.matmul(
                    fs, ones_f, sumacc[:, qb * NB:(qb + 1) * NB],
                    start=True, stop=True)
                nc.vector.tensor_copy(
                    out=sums_sb[:, qb * NB:(qb + 1) * NB], in_=fs)
            nc.sync.dma_start(out=sums[:, :], in_=sums_sb)

            # D: y^T[e,q] -> bf16 (halves the output DMA). qb-major so D's
            # qb=0 sweep runs while C's qb=1 copies drain, and every store
            # is small ([128,512]) for a short tail.
            with tc.tile_pool(name="yt_sb", bufs=4) as yt_sb:
                for qb in range(Q // NB):
                    for e in range(DT):
                        ysb = yt_sb.tile([P, NB], BF16, tag="y")
                        ps = scr_ps.tile([P, NB], F32, tag="scr")
                        for d in range(DT):
                            nc.tensor.matmul(
                                ps, wp_sb[:, d * D + e * P:d * D + e * P + P],
                                out_t[d][:, qb * NB:(qb + 1) * NB],
                                start=(d == 0), stop=(d == DT - 1))
                        nc.vector.tensor_copy(out=ysb, in_=ps)
                        nc.sync.dma_start(
                            out=yt[:, e * D + qb * NB:e * D + (qb + 1) * NB],
                            in_=ysb)

    return nc


_NC_CACHE = None


def _get_nc():
    global _NC_CACHE
    if _NC_CACHE is None:
        _NC_CACHE = build_nc()
    return _NC_CACHE


# ---------------------------------------------------------------------------
# Host side
# ---------------------------------------------------------------------------


def _pack_img(a):
    """[1024, 1024] operand -> [128, 8192] SBUF image (bf16)."""
    # img[p, d*1024 + c] = a[d*128 + p, c]
    bf = ml_dtypes.bfloat16
    return np.ascontiguousarray(
        a.reshape(DT, P, D).transpose(1, 0, 2).reshape(P, W)).astype(bf)


def _prep_in_maps(x, W_qkv, b_qkv, W_proj, b_proj):
    x = np.asarray(x, dtype=np.float32)
    W_qkv = np.asarray(W_qkv, dtype=np.float32)
    b_qkv = np.asarray(b_qkv, dtype=np.float32)
    W_proj = np.asarray(W_proj, dtype=np.float32)
    b_proj = np.asarray(b_proj, dtype=np.float32)

    scale = 1.0 / math.sqrt(D)
    wq_h = _pack_img((W_qkv[:D] * scale).T)
    wk_h = _pack_img(W_qkv[D:2 * D].T)
    wv_h = _pack_img(W_qkv[2 * D:].T)
    wp_h = _pack_img(W_proj.T)
    bqk_h = np.ascontiguousarray(
        np.concatenate([b_qkv[:D] * scale, b_qkv[D:2 * D]])
        .reshape(2 * DT, P).T).astype(np.float32)
    b_eff = b_proj + W_proj @ b_qkv[2 * D:]

    in_maps = []
    for c in range(N_CORES):
        b, h = divmod(c, 2)
        xt_h = _pack_img(x[b, h * H:(h + 1) * H, :].T)
        in_maps.append({"xt": xt_h, "wq": wq_h, "wk": wk_h, "wv": wv_h,
                        "wp": wp_h, "bqk": bqk_h})
    return in_maps, b_eff


def _postprocess(results, b_eff):
    y = np.empty((4, S, D), dtype=np.float32)
    for c in range(N_CORES):
        b, h = divmod(c, 2)
        # yt image [128, 8192] -> y^T [1024(e), 1024(q)]
        yimg = results[c]["yt"].astype(np.float32)
        ytc = yimg.reshape(P, DT, D).transpose(1, 0, 2).reshape(D, Q)
        sc = results[c]["sums"][0]      # [Q] softmax denominators
        y[b, h * Q:(h + 1) * Q, :] = ytc.T / sc[:, None] + b_eff[None, :]
    return y


def kernel(x, W_qkv, b_qkv, W_proj, b_proj, **run_kwargs):
    nc = _get_nc()
    in_maps, b_eff = _prep_in_maps(x, W_qkv, b_qkv, W_proj, b_proj)
    last_exc = None
    for attempt in range(3):
        try:
            res = run_bass_kernel_spmd(nc, in_maps,
                                       core_ids=list(range(N_CORES)),
                                       **run_kwargs)
            break
        except Exception as exc:  # transient NRT device errors
            last_exc = exc
            import time
            time.sleep(2.0 * (attempt + 1))
    else:
        raise last_exc
    y = _postprocess(res.results, b_eff)
    kernel.last_result = res
    return y


# revision 22
# speedup vs baseline: 1.1776x; 1.1776x over previous
"""Trainium2 Bass kernel for a single-head attention layer.

Problem: x [4, 2048, 1024] f32; torch-Linear qkv (W_qkv [3072, 1024]) ->
single-head attention (d=1024) -> output projection (W_proj [1024, 1024]).

Sharding: 8 NeuronCores = 4 batches x 2 query-halves. Each core computes
attention for 1024 queries of one batch. K^T/V are computed only for the
core's own 1024-key half; the partner half is exchanged through pairwise
AllGather collectives (replica groups [[0,1],[2,3],[4,5],[6,7]]). While the
collectives are in flight the core projects Q and runs attention over its own
keys; the partner half is imported from the gathered DRAM buffer with a
partition-id-derived dynamic row offset (rank parity picks the block).
Per-core key order is [own half, partner half] - softmax is permutation-
invariant over keys, so this is safe.

Host-side folds (all exact or fp32):
  - 1/sqrt(d) scale folded into W_q / b_q
  - V-bias folded through the projection: b_eff = b_proj + W_proj @ b_v
  - softmax normalization on host: the device returns unnormalized
    y^T = W_proj @ (exp(scores^T) @ V)^T plus per-query exp-sums.
    No max-subtraction needed: logits are ~N(0,1), exp is safe in f32.

DMA strategy: every input is pre-packed on the host into its exact SBUF
image [128, 8192] so each dma_start moves a [128, 2048] slice with fully
contiguous 4KB rows (the sweet spot for per-stream DGE throughput; one
dma_start fans its packets across all 16 DMA engines). This cuts the input
load from 33 issue-serialized 1KB-row dma_starts to 17 4KB-row ones.

K^T is computed in two contraction passes (d0-3 -> f32 stash via the scalar
engine with the K-bias folded in, then d4-7 + stash via a DVE add) so the
tensor engine starts after only 2MB of input has landed instead of 4MB.

Device program (per core; all matmuls bf16 with f32 PSUM accumulation):
  A: K^T_own[e,s] -> SBUF + send; V_own[s,e] -> SBUF + send; Q^T[e,q]
  B: scores^T[j,q] = K^T-stationary x Q^T-moving -> Exp -> SBUF bf16;
     per-q sums via ones-column matmul (keys on partitions)
  C: out^T[d,q] = V-stationary x exp^T-moving (accumulated over key tiles)
  D: y^T[e,q] = W_proj^T-stationary x out^T-moving -> bf16 out
"""

import math

import numpy as np
import ml_dtypes

import concourse.bass as bass
import concourse.tile as tile
from concourse import mybir
from concourse.bass_utils import run_bass_kernel_spmd
from concourse.vector_clock import ScopedClock, VectorClock

BF16 = mybir.dt.bfloat16
F32 = mybir.dt.float32
AF = mybir.ActivationFunctionType

D = 1024   # model dim
S = 2048   # sequence length
Q = 1024   # queries per core
H = 1024   # keys per core (own half)
P = 128    # SBUF partitions
NB = 512   # matmul moving-block size
DT = D // P
HT = H // P
ST = S // P
N_CORES = 8
GROUPS = [[0, 1], [2, 3], [4, 5], [6, 7]]
W = D * DT          # 8192: packed image width
LQ = 2048           # dma_start slice width (4KB bf16 rows)

# ---------------------------------------------------------------------------
# Workarounds for this container's walrus, which rejects any instruction
# carrying more than one sem wait ("Too many sync wait commands").
# ---------------------------------------------------------------------------


def _patched_drain_and_barrier(self, tick_clock, wait_clock):
    # Split the kernel-tail drain into one drain per semaphore (1 wait each).
    gc = tick_clock.global_clock
    n = len(gc)
    for i in range(n):
        if gc[i] > 0:
            vec = [0] * n
            vec[i] = gc[i]
            dr = self.nc.sync.drain()
            wait_clock.add_sem_waits(dr.ins, ScopedClock({None: VectorClock(vec)}))
    self.nc.all_engine_barrier()
    popped = self.nc._tile_sem_poison_stack.pop()
    assert popped is self._sem_poison
    self.nc.clear_and_free_semaphores(list(self.sems.allocated().values()))
    self.nc.all_engine_barrier()


_MAX_WAITS = 1
_split_counter = [0]


def _split_excess_waits(ordered):
    # Hoist excess waits onto preceding same-engine NoOps.
    for insts in ordered.values():
        new_list = []
        for inst in insts:
            si = inst.sync_info
            waits = list(si.on_wait) if si is not None and si.on_wait else []
            if len(waits) > _MAX_WAITS and inst.engine is not None:
                extra, keep = waits[:-_MAX_WAITS], waits[-_MAX_WAITS:]
                for w in extra:
                    _split_counter[0] += 1
                    nop = mybir.InstNoOp(
                        name=f"waitsplit-{_split_counter[0]}",
                        sync_info=mybir.SyncInfo(on_wait=[w], on_update=[]),
                        bass_nofuse=True,
                        engine=inst.engine,
                    )
                    new_list.append(nop)
                inst.sync_info = mybir.SyncInfo(
                    on_wait=keep, on_update=list(si.on_update))
            new_list.append(inst)
        insts[:] = new_list


def _install_patches():
    if getattr(tile.TileContext, "_attn_patched", False):
        return
    tile.TileContext._drain_and_barrier = _patched_drain_and_barrier
    orig_lower = tile.TileContext._lower_ordered_insts

    def _lower_with_wait_split(self, ordered):
        _split_excess_waits(ordered)
        return orig_lower(self, ordered)

    tile.TileContext._lower_ordered_insts = _lower_with_wait_split
    tile.TileContext._attn_patched = True


_install_patches()

# ---------------------------------------------------------------------------
# Device program
# ---------------------------------------------------------------------------


def build_nc():
    nc = bass.Bass("TRN2", target_bir_lowering=False, debug=False,
                   num_devices=N_CORES)

    # Packed SBUF images: col block d*1024+c holds row d*128+p, col c of the
    # logical [1024, 1024] operand.
    xt = nc.dram_tensor("xt", [P, W], BF16, kind="ExternalInput").ap()
    wq = nc.dram_tensor("wq", [P, W], BF16, kind="ExternalInput").ap()
    wk = nc.dram_tensor("wk", [P, W], BF16, kind="ExternalInput").ap()
    wv = nc.dram_tensor("wv", [P, W], BF16, kind="ExternalInput").ap()
    wp = nc.dram_tensor("wp", [P, W], BF16, kind="ExternalInput").ap()
    bqk = nc.dram_tensor("bqk", [P, 2 * DT], F32, kind="ExternalInput").ap()
    yt = nc.dram_tensor("yt", [P, W], BF16, kind="ExternalOutput").ap()
    sums = nc.dram_tensor("sums", [1, Q], F32, kind="ExternalOutput").ap()

    # K is exchanged in two halves (e0-3, e4-7) so the partner import can
    # start as soon as the first half of K^T is computed.
    k_send = [nc.dram_tensor(f"k_send{h}", [D // 2, H], BF16).ap()
              for h in range(2)]
    k_recv = [nc.dram_tensor(f"k_recv{h}", [D, H], BF16).ap()
              for h in range(2)]
    v_send = nc.dram_tensor("v_send", [H, D], BF16).ap()
    v_recv = nc.dram_tensor("v_recv", [2 * H, D], BF16).ap()

    from contextlib import ExitStack
    with tile.TileContext(nc) as tc, ExitStack() as stack:
        res = stack.enter_context(tc.tile_pool(name="res", bufs=1))
        qt_t = [res.tile([P, Q], BF16, tag=f"qt{e}", name=f"qt{e}")
                for e in range(DT)]
        kt_own = [res.tile([P, H], BF16, tag=f"kto{e}", name=f"kto{e}")
                  for e in range(DT)]
        v_own = [res.tile([P, D], BF16, tag=f"vo{j}", name=f"vo{j}")
                 for j in range(HT)]
        bias_t = res.tile([P, 2 * DT], F32, tag="bias", name="bias")

        # ---------------- Phase A: projections + exchange ----------------
        # pha_ps uses 3 banks (+1 warmup) so phase B's pools land on virgin
        # PSUM banks - otherwise B's first matmul waits phase A's last drain.
        with tc.tile_pool(name="pha", bufs=1) as pha, \
             tc.tile_pool(name="pha_ps", bufs=3, space="PSUM") as pha_ps:
            xt_sb = pha.tile([P, W], BF16, tag="xt", name="xt_sb")
            wk_sb = pha.tile([P, W], BF16, tag="wk", name="wk_sb")
            wv_sb = pha.tile([P, W], BF16, tag="wv", name="wv_sb")
            wq_sb = pha.tile([P, W], BF16, tag="wq", name="wq_sb")
            # f32 stash for the partial K^T contractions (+ K-bias)
            stash = [pha.tile([P, NB], F32, tag=f"st{g}", name=f"st{g}")
                     for g in range(2 * DT)]
            # PE warmup: junk matmuls on a zeroed tile, gated only by a DVE
            # memset - they run during the input-DMA wait and ramp the tensor
            # clock to full p-state before the first real matmul.
            warm = pha.tile([P, NB], BF16, tag="warm", name="warm")
            nc.vector.memset(warm, 0.0)

            def qsl(t, q4):
                return t[:, q4 * LQ:(q4 + 1) * LQ]

            # Inputs split across two DGE queues so both rings spin up
            # concurrently and the 1MB pass-1 gate (xt q0 on sync, wk q0 on
            # scalar) issues ~1.5us earlier. wk fits on the scalar queue:
            # its 4 issues retire before phase A's first scalar drain.
            nc.sync.dma_start(out=bias_t, in_=bqk[:, :])
            for q4 in range(4):
                nc.scalar.dma_start(out=qsl(wk_sb, q4), in_=qsl(wk, q4))
            for q4 in range(4):
                nc.sync.dma_start(out=qsl(xt_sb, q4), in_=qsl(xt, q4))
            for q4 in range(4):
                nc.sync.dma_start(out=qsl(wv_sb, q4), in_=qsl(wv, q4))
            for q4 in range(4):
                nc.sync.dma_start(out=qsl(wq_sb, q4), in_=qsl(wq, q4))

            # Dummy activation pulls the ~1.5us ACT_TABLE_LOAD into the
            # DMA-wait window (it otherwise delays pass-1's first drain).
            actwarm = pha.tile([P, 1], F32, tag="actw", name="actwarm")
            nc.vector.memset(actwarm, 0.0)
            nc.scalar.activation(out=actwarm, in_=actwarm, func=AF.Exp)

            with tc.tile_pool(name="wm_ps", bufs=1, space="PSUM") as wm_pool:
                wm_ps = wm_pool.tile([P, NB], F32, tag="wm", name="wm_ps")
                for _ in range(14):
                    nc.tensor.matmul(wm_ps, warm[:, :P], warm,
                                     start=True, stop=True)
                nc.vector.tensor_copy(out=warm, in_=wm_ps)

            def xcol(d, c0, n):
                return xt_sb[:, d * D + c0:d * D + c0 + n]

            def wcol(t, d, c0, n):
                return t[:, d * D + c0:d * D + c0 + n]

            # K^T_own[e, s_own] in two contraction passes: d0-1 -> f32 stash
            # (+K-bias) so the PE starts after 1MB of input, then d2-7 +
            # stash -> bf16. Pass-1 drains alternate between the scalar and
            # DVE engines so the 2-matmul chains stay PE-bound.
            for g in range(2 * DT):
                e, sb = divmod(g, 2)
                ps = pha_ps.tile([P, NB], F32, tag="ps")
                for d in range(2):
                    nc.tensor.matmul(
                        ps, wcol(wk_sb, d, e * P, P),
                        xcol(d, sb * NB, NB),
                        start=(d == 0), stop=(d == 1))
                if g % 2 == 0:
                    nc.scalar.activation(
                        out=stash[g], in_=ps,
                        func=AF.Identity, bias=bias_t[:, DT + e:DT + e + 1])
                else:
                    nc.vector.tensor_scalar_add(
                        stash[g], ps, bias_t[:, DT + e:DT + e + 1])
            for h in range(2):
                for e in range(h * 4, h * 4 + 4):
                    for sb in range(2):
                        ps = pha_ps.tile([P, NB], F32, tag="ps")
                        for d in range(2, DT):
                            nc.tensor.matmul(
                                ps, wcol(wk_sb, d, e * P, P),
                                xcol(d, sb * NB, NB),
                                start=(d == 2), stop=(d == DT - 1))
                        nc.vector.tensor_add(
                            kt_own[e][:, sb * NB:(sb + 1) * NB],
                            ps, stash[e * 2 + sb])
                    nc.sync.dma_start(
                        out=k_send[h][(e - h * 4) * P:(e - h * 4 + 1) * P, :],
                        in_=kt_own[e])
                nc.gpsimd.collective_compute(
                    "AllGather", mybir.AluOpType.bypass,
                    replica_groups=GROUPS,
                    ins=[k_send[h][:, :]], outs=[k_recv[h][:, :]])

            # V_own[s_own, e] -> SBUF (kept) + v_send. V before Q so the V
            # AllGather triggers early enough for the phase-C j=8 boundary.
            for j in range(HT):
                for eb in range(D // NB):
                    ps = pha_ps.tile([P, NB], F32, tag="ps")
                    for d in range(DT):
                        nc.tensor.matmul(
                            ps, xcol(d, j * P, P),
                            wcol(wv_sb, d, eb * NB, NB),
                            start=(d == 0), stop=(d == DT - 1))
                    nc.vector.tensor_copy(
                        out=v_own[j][:, eb * NB:(eb + 1) * NB], in_=ps)
                nc.sync.dma_start(out=v_send[j * P:(j + 1) * P, :],
                                  in_=v_own[j])

            nc.gpsimd.collective_compute(
                "AllGather", mybir.AluOpType.bypass, replica_groups=GROUPS,
                ins=[v_send[:, :]], outs=[v_recv[:, :]])

            # Q^T[e, q] (overlaps the collectives)
            for e in range(DT):
                for qb in range(Q // NB):
                    ps = pha_ps.tile([P, NB], F32, tag="ps")
                    for d in range(DT):
                        nc.tensor.matmul(
                            ps, wcol(wq_sb, d, e * P, P),
                            xcol(d, qb * NB, NB),
                            start=(d == 0), stop=(d == DT - 1))
                    nc.scalar.activation(
                        out=qt_t[e][:, qb * NB:(qb + 1) * NB], in_=ps,
                        func=AF.Identity, bias=bias_t[:, e:e + 1])

        # ---------------- Phases B, C, D ----------------
        with tc.tile_pool(name="phb", bufs=1) as phb, \
             tc.tile_pool(name="scr_ps", bufs=2, space="PSUM") as scr_ps, \
             tc.tile_pool(name="sum_ps", bufs=1, space="PSUM") as sum_ps, \
             tc.tile_pool(name="av_ps", bufs=1, space="PSUM") as av_ps:
            # Partner-half import: rank parity picks the gathered block.
            # One dynamic-offset DMA per tensor (SP base registers are scarce).
            # Partner imports on the sync queue (behind the v_send stores).
            # The K AllGather halves complete long before the j=8 boundary;
            # the gpsimd queue is NOT usable here because collective_compute
            # blocks it until the collective completes.
            pid = nc.sync.partition_id()
            parity = pid % 2
            kbase = (1 - parity) * (D // 2)
            pbase = (1 - parity) * H
            kt_par = phb.tile([P, DT, H], BF16, tag="ktp", name="ktp")
            v_par = phb.tile([P, HT, D], BF16, tag="vp", name="vp")
            for h in range(2):
                nc.sync.dma_start(
                    out=kt_par[:, h * 4:(h + 1) * 4, :],
                    in_=k_recv[h][bass.ds(kbase, D // 2), :].rearrange(
                        "(e p) s -> p e s", p=P))
            nc.sync.dma_start(
                out=v_par,
                in_=v_recv[bass.ds(pbase, H), :].rearrange(
                    "(j p) e -> p j e", p=P))

            exp_t = [phb.tile([P, Q], BF16, tag=f"exp{j}", name=f"exp{j}")
                     for j in range(ST)]
            out_t = [phb.tile([P, Q], BF16, tag=f"out{j}", name=f"out{j}")
                     for j in range(DT)]
            wp_sb = phb.tile([P, W], BF16, tag="wp", name="wp_sb")
            for q4 in range(4):
                nc.sync.dma_start(out=wp_sb[:, q4 * LQ:(q4 + 1) * LQ],
                                  in_=wp[:, q4 * LQ:(q4 + 1) * LQ])

            # f32 per-key-lane partial sums, accumulated on the (idle) DVE;
            # reduced across partitions with two small f32 matmuls at the end.
            sumacc = phb.tile([P, Q], F32, tag="sumacc", name="sumacc")
            ones_f = phb.tile([P, 1], F32, tag="ones_f", name="ones_f")
            nc.vector.memset(ones_f, 1.0)

            def kt_slice(e, j):
                if j < HT:
                    return kt_own[e][:, j * P:(j + 1) * P]
                return kt_par[:, e, (j - HT) * P:(j - HT + 1) * P]

            def v_tile(j):
                if j < HT:
                    return v_own[j]
                return v_par[:, j - HT, :]

            # B: scores^T[j,q] -> exp -> sums (own keys first)
            for j in range(ST):
                for qb in range(Q // NB):
                    ps = scr_ps.tile([P, NB], F32, tag="scr")
                    for e in range(DT):
                        nc.tensor.matmul(
                            ps, kt_slice(e, j),
                            qt_t[e][:, qb * NB:(qb + 1) * NB],
                            start=(e == 0), stop=(e == DT - 1))
                    nc.scalar.activation(
                        out=exp_t[j][:, qb * NB:(qb + 1) * NB], in_=ps,
                        func=AF.Exp)
                    sl = slice(qb * NB, (qb + 1) * NB)
                    if j == 0:
                        nc.vector.tensor_copy(
                            out=sumacc[:, sl], in_=exp_t[j][:, sl])
                    else:
                        nc.vector.tensor_add(
                            sumacc[:, sl], sumacc[:, sl], exp_t[j][:, sl])

            # C: out^T[d',q] accumulated over all 16 key tiles. Each PSUM
            # bank runs its full j-sweep serially, so bank i's drain copy
            # overlaps bank i+1's sweep - no stall when the bank is reused.
            for g in range(2):
                for qb in range(Q // NB):
                    for i in range(4):
                        dp = g * 4 + i
                        ps_o = av_ps.tile([P, NB], F32, tag=f"av{i}",
                                          name=f"avps{i}")
                        for j in range(ST):
                            nc.tensor.matmul(
                                ps_o, v_tile(j)[:, dp * P:(dp + 1) * P],
                                exp_t[j][:, qb * NB:(qb + 1) * NB],
                                start=(j == 0), stop=(j == ST - 1))
                        nc.vector.tensor_copy(
                            out=out_t[dp][:, qb * NB:(qb + 1) * NB],
                            in_=ps_o)

            # Softmax denominators: reduced after phase C so the tiny fsum
            # matmuls never stall the B->C transition on the PE queue.
            sums_sb = phb.tile([1, Q], F32, tag="sums_sb")
            for qb in range(Q // NB):
                fs = sum_ps.tile([1, NB], F32, tag=f"fsum{qb}",
                                 name=f"fsum{qb}")
                nc.tensor.matmul(
                    fs, ones_f, sumacc[:, qb * NB:(qb + 1) * NB],
                    start=True, stop=True)
                nc.vector.tensor_copy(
                    out=sums_sb[:, qb * NB:(qb + 1) * NB], in_=fs)
            nc.sync.dma_start(out=sums[:, :], in_=sums_sb)

            # D: y^T[e,q] -> bf16 (halves the output DMA). qb-major so D's
            # qb=0 sweep runs while C's qb=1 copies drain, and every store
            # is small ([128,512]) for a short tail.
            with tc.tile_pool(name="yt_sb", bufs=4) as yt_sb:
                for qb in range(Q // NB):
                    for e in range(DT):
                        ysb = yt_sb.tile([P, NB], BF16, tag="y")
                        ps = scr_ps.tile([P, NB], F32, tag="scr")
                        for d in range(DT):
                            nc.tensor.matmul(
                                ps, wp_sb[:, d * D + e * P:d * D + e * P + P],
                                out_t[d][:, qb * NB:(qb + 1) * NB],
                                start=(d == 0), stop=(d == DT - 1))
                        nc.vector.tensor_copy(out=ysb, in_=ps)
                        nc.sync.dma_start(
                            out=yt[:, e * D + qb * NB:e * D + (qb + 1) * NB],
                            in_=ysb)

    return nc


_NC_CACHE = None


def _get_nc():
    global _NC_CACHE
    if _NC_CACHE is None:
        _NC_CACHE = build_nc()
    return _NC_CACHE


# ---------------------------------------------------------------------------
# Host side
# ---------------------------------------------------------------------------


def _pack_img(a):
    """[1024, 1024] operand -> [128, 8192] SBUF image (bf16)."""
    # img[p, d*1024 + c] = a[d*128 + p, c]
    bf = ml_dtypes.bfloat16
    return np.ascontiguousarray(
        a.reshape(DT, P, D).transpose(1, 0, 2).reshape(P, W)).astype(bf)


def _prep_in_maps(x, W_qkv, b_qkv, W_proj, b_proj):
    x = np.asarray(x, dtype=np.float32)
    W_qkv = np.asarray(W_qkv, dtype=np.float32)
    b_qkv = np.asarray(b_qkv, dtype=np.float32)
    W_proj = np.asarray(W_proj, dtype=np.float32)
    b_proj = np.asarray(b_proj, dtype=np.float32)

    scale = 1.0 / math.sqrt(D)
    wq_h = _pack_img((W_qkv[:D] * scale).T)
    wk_h = _pack_img(W_qkv[D:2 * D].T)
    wv_h = _pack_img(W_qkv[2 * D:].T)
    wp_h = _pack_img(W_proj.T)
    bqk_h = np.ascontiguousarray(
        np.concatenate([b_qkv[:D] * scale, b_qkv[D:2 * D]])
        .reshape(2 * DT, P).T).astype(np.float32)
    b_eff = b_proj + W_proj @ b_qkv[2 * D:]

    in_maps = []
    for c in range(N_CORES):
        b, h = divmod(c, 2)
        xt_h = _pack_img(x[b, h * H:(h + 1) * H, :].T)
        in_maps.append({"xt": xt_h, "wq": wq_h, "wk": wk_h, "wv": wv_h,
                        "wp": wp_h, "bqk": bqk_h})
    return in_maps, b_eff


def _postprocess(results, b_eff):
    y = np.empty((4, S, D), dtype=np.float32)
    for c in range(N_CORES):
        b, h = divmod(c, 2)
        # yt image [128, 8192] -> y^T [1024(e), 1024(q)]
        yimg = results[c]["yt"].astype(np.float32)
        ytc = yimg.reshape(P, DT, D).transpose(1, 0, 2).reshape(D, Q)
        sc = results[c]["sums"][0]      # [Q] softmax denominators
        y[b, h * Q:(h + 1) * Q, :] = ytc.T / sc[:, None] + b_eff[None, :]
    return y


def kernel(x, W_qkv, b_qkv, W_proj, b_proj, **run_kwargs):
    nc = _get_nc()
    in_maps, b_eff = _prep_in_maps(x, W_qkv, b_qkv, W_proj, b_proj)
    last_exc = None
    for attempt in range(3):
        try:
            res = run_bass_kernel_spmd(nc, in_maps,
                                       core_ids=list(range(N_CORES)),
                                       **run_kwargs)
            break
        except Exception as exc:  # transient NRT device errors
            last_exc = exc
            import time
            time.sleep(2.0 * (attempt + 1))
    else:
        raise last_exc
    y = _postprocess(res.results, b_eff)
    kernel.last_result = res
    return y
